# revision 1
# baseline (speedup 1.0000x reference)
"""CrossVariableAttention Bass/Tile kernel for TRN2.

Per-core program (data parallel over batch, one batch element per core).
Two host-side algebraic fusions cut the on-chip matmul count from 832 to 720:

  scores:  S = (q+bq')(k+bk)^T.  Terms constant along the softmax axis (m)
           cancel, so with B := Wq'.Wk^T (host) and u := Wk.bq' (host):
             S^T[m,n] ~ X[:,m]^T.B^T.X[:,n] + r[m],  r = u^T.X
           C := B^T.X is one 512x512x2048 matmul; r folds into the exp bias.
  output:  (P.V).Wp = P.(V.Wp), and V.Wp = X^T.(Wv.Wp) + bv.Wp, so with
           Wvp := Wv.Wp and bvp := bv.Wp (host):
             VP := X^T.Wvp + bvp;  y_un^T = VP^T.Pt;  y = y_un*recip + bp
           The bvp term is exact after softmax normalization (sum_m P = 1).

Per-core dataflow (all matmuls in float32r = full-rate fp32, ~1e-4 rel err):
  phase 1:  C [d,n] = WB^T.X;  r [1,n] = u^T.X;  VP [m,d] = X^T.Wvp + bvp
  phase 2, per slab of 512 queries:
    S^T = X^T.C        [m, 512]   (16 psum tiles)
    Pt  = exp(S^T + r[m])         (ACT, bias per partition)
    den = ones^T.Pt    [1, 512]   (16 M=1 matmuls, accumulated)
    recip_bc = 1/den broadcast to [128, 512]  (DRAM bounce)
    Y^T = VP^T.Pt      [dout, 512]
    y   = Y^T*recip_bc + bp  -> DMA out ([L, N] layout, no transposes anywhere)
"""

from contextlib import ExitStack

import concourse.bass as bass
import concourse.mybir as mybir
import concourse.tile as tile
from concourse.bass import ds
from concourse.vector_clock import ScopedClock

F32 = mybir.dt.float32
F32R = mybir.dt.float32r
AF = mybir.ActivationFunctionType

P = 128
D = 512
N = 2048
DCH = D // P         # 4 d chunks
NCH = N // P         # 16 token chunks (m)
NSLAB = N // 512     # 4 query slabs


# ---------------------------------------------------------------------------
# The walrus build in this env accepts at most ONE sync wait per instruction
# (setupSyncWait: "Too many sync wait commands").  Tile attaches several.
# Fix: split excess waits onto engine-local NOPs placed just before the
# instruction (same engine => same stream order => identical semantics).
MAX_WAITS_PER_INST = 1


class SplitDrainTileContext(tile.TileContext):
    def _drain_and_barrier(self, tick_clock, wait_clock):
        nc = self.nc
        probe = nc.sync.nop(nofuse=True, hint="split_drain_waits")
        wait_clock.add_sem_waits(
            probe.ins, ScopedClock({None: tick_clock.global_clock})
        )
        waits = list(probe.ins.sync_info.on_wait)
        probe.ins.sync_info.on_wait = waits[:MAX_WAITS_PER_INST]
        for i in range(MAX_WAITS_PER_INST, len(waits), MAX_WAITS_PER_INST):
            extra = nc.sync.nop(nofuse=True, hint="split_drain_waits")
            extra.ins.sync_info = mybir.SyncInfo(
                on_wait=waits[i : i + MAX_WAITS_PER_INST], on_update=[]
            )
        nc.sync.drain()
        nc.all_engine_barrier()
        assert self.sems is not None
        popped = nc._tile_sem_poison_stack.pop()
        assert popped is self._sem_poison
        nc.clear_and_free_semaphores(list(self.sems.allocated().values()))
        nc.all_engine_barrier()


def split_sync_waits(nc, max_waits=MAX_WAITS_PER_INST):
    n_split = 0
    for fn in nc.m.functions:
        for bb in fn.blocks:
            insts = list(bb.instructions)
            out = []
            changed = False
            for inst in insts:
                si = getattr(inst, "sync_info", None)
                if si is not None:
                    waits = list(si.on_wait or [])
                    if len(waits) > max_waits:
                        changed = True
                        for j, w in enumerate(waits[: len(waits) - max_waits]):
                            out.append(
                                mybir.InstNoOp(
                                    name=f"{inst.name}-sw{j}",
                                    engine=inst.engine,
                                    bass_nofuse=True,
                                    sync_info=mybir.SyncInfo(
                                        on_wait=[w], on_update=[]
                                    ),
                                )
                            )
                            n_split += 1
                        si.on_wait = waits[len(waits) - max_waits :]
                out.append(inst)
            if changed:
                bb.instructions = out
    return n_split


def build_nc():
    nc = bass.Bass()

    x = nc.declare_dram_parameter("x", [D, N], F32R, isOutput=False)
    wb = nc.declare_dram_parameter("wb", [DCH, P, DCH, P], F32R, isOutput=False)
    wvp = nc.declare_dram_parameter("wvp", [D, D], F32R, isOutput=False)
    u = nc.declare_dram_parameter("u", [D], F32R, isOutput=False)
    ones_in = nc.declare_dram_parameter("ones", [P, 1], F32R, isOutput=False)
    bvp = nc.declare_dram_parameter("bvp", [D], F32, isOutput=False)
    bp = nc.declare_dram_parameter("bp", [D], F32, isOutput=False)
    y = nc.declare_dram_parameter("y", [D, N], F32, isOutput=True)
    r_dram = nc.dram_tensor("r_scratch", [N], F32)
    recip_dram = nc.dram_tensor("recip_scratch", [NSLAB, 512], F32)

    with SplitDrainTileContext(nc) as tc, ExitStack() as ctx:
        consts = ctx.enter_context(tc.tile_pool(name="consts", bufs=1))
        big = ctx.enter_context(tc.tile_pool(name="big", bufs=1))
        small = ctx.enter_context(tc.tile_pool(name="small", bufs=3))

        bp_sb = consts.tile([P, DCH], F32, tag="bp")
        u_sb = consts.tile([P, DCH], F32R, tag="u")
        ones = consts.tile([P, 1], F32R, tag="ones")
        bvp_bc = consts.tile([P, D], F32, tag="bvp")
        wvp_sb = consts.tile([P, DCH, D], F32R, tag="wvp")
        rcol_sb = consts.tile([P, NCH], F32, tag="rcol")

        # --- persistent big tensors --------------------------------------
        c_sb = big.tile([P, DCH, N], F32R, tag="c")
        vp_sb = big.tile([P, NCH, D], F32R, tag="vp")
        x_tiles = []
        for nb in range(NSLAB):
            xt_nb = big.tile([P, DCH, 512], F32R, tag=f"x{nb}")
            x_tiles.append(xt_nb)

        # --- phase 1: C, r, VP (input DMAs just-in-time) ------------------
        with tc.tile_pool(name="xin", bufs=1) as xin, \
             tc.tile_pool(name="ps1", bufs=4, space="PSUM") as ps1, \
             tc.tile_pool(name="ps_r", bufs=2, space="PSUM") as ps_r:
            wb_tiles = []
            for oc in range(DCH):
                wbt = xin.tile([P, DCH, P], F32R, tag=f"wb{oc}")
                wb_tiles.append(wbt)
            nc.sync.dma_start(out=wb_tiles[0], in_=wb[0])
            nc.sync.dma_start(out=u_sb, in_=u.rearrange("(c p) -> p c", p=P))
            x_re = x.rearrange("(c p) n -> p c n", p=P)
            nc.sync.dma_start(out=x_tiles[0], in_=x_re[:, :, ds(0, 512)])
            for oc in range(1, DCH):
                nc.sync.dma_start(out=wb_tiles[oc], in_=wb[oc])
            for nb in range(1, NSLAB):
                nc.sync.dma_start(
                    out=x_tiles[nb], in_=x_re[:, :, ds(nb * 512, 512)]
                )
            nc.sync.dma_start(out=wvp_sb, in_=wvp.rearrange("(c p) o -> p c o", p=P))
            nc.sync.dma_start(out=bp_sb, in_=bp.rearrange("(c p) -> p c", p=P))
            nc.sync.dma_start(out=ones, in_=ones_in[:, :])
            bvp_ap = bvp[:]
            nc.sync.dma_start(
                out=bvp_bc,
                in_=bass.AP(
                    tensor=bvp_ap.tensor, offset=bvp_ap.offset,
                    ap=[[0, P], bvp_ap.ap[0]],
                ),
            )

            for nb in range(NSLAB):
                # C[:, :, slab] = WB^T . X[:, slab]
                for oc in range(DCH):
                    ps = ps1.tile([P, 512], F32, tag="ps1")
                    for ic in range(DCH):
                        nc.tensor.matmul(
                            ps,
                            wb_tiles[oc][:, ic, :],
                            x_tiles[nb][:, ic, :],
                            start=(ic == 0),
                            stop=(ic == DCH - 1),
                        )
                    nc.scalar.copy(out=c_sb[:, oc, ds(nb * 512, 512)], in_=ps)
                # r[slab] = u^T . X[:, slab]
                psr = ps_r.tile([1, 512], F32, tag="psr")
                for ic in range(DCH):
                    nc.tensor.matmul(
                        psr,
                        u_sb[:, ic : ic + 1],
                        x_tiles[nb][:, ic, :],
                        start=(ic == 0),
                        stop=(ic == DCH - 1),
                    )
                r_sb = small.tile([1, 512], F32, tag="rsb")
                nc.vector.tensor_copy(out=r_sb, in_=psr)
                nc.sync.dma_start(out=r_dram[ds(nb * 512, 512)], in_=r_sb)

            # VP = X^T . Wvp + bvp
            for mc in range(NCH):
                ps = ps1.tile([P, 512], F32, tag="ps1")
                for ic in range(DCH):
                    nc.tensor.matmul(
                        ps,
                        x_tiles[mc // 4][:, ic, ds((mc % 4) * P, P)],
                        wvp_sb[:, ic, :],
                        start=(ic == 0),
                        stop=(ic == DCH - 1),
                    )
                nc.vector.tensor_add(out=vp_sb[:, mc, :], in0=ps, in1=bvp_bc)

            # r in column layout [128, 16]: rcol[p, mc] = r[mc*128 + p]
            nc.sync.dma_start(
                out=rcol_sb, in_=r_dram.rearrange("(c p) -> p c", p=P)
            )

        # --- phase 2: attention, per slab of 512 queries ------------------
        with tc.tile_pool(name="pt", bufs=20) as pt_pool, \
             tc.tile_pool(name="outp", bufs=4) as outp, \
             tc.tile_pool(name="ps_st", bufs=4, space="PSUM") as ps_st, \
             tc.tile_pool(name="ps_den", bufs=1, space="PSUM") as ps_den, \
             tc.tile_pool(name="ps_y", bufs=3, space="PSUM") as ps_y:
            for nb in range(NSLAB):
                nsl = ds(nb * 512, 512)

                # S^T tiles + exp(S + r); DVE accumulates Pt pairs so the
                # denominator needs only ONE M=1 matmul instead of 16
                pt_tiles = []
                acc = small.tile([P, 512], F32R, tag="denacc")
                for mc in range(NCH):
                    ps = ps_st.tile([P, 512], F32, tag="st")
                    for ic in range(DCH):
                        nc.tensor.matmul(
                            ps,
                            x_tiles[mc // 4][:, ic, ds((mc % 4) * P, P)],
                            c_sb[:, ic, nsl],
                            start=(ic == 0),
                            stop=(ic == DCH - 1),
                        )
                    pt = pt_pool.tile([P, 512], F32R, tag="pt")
                    nc.scalar.activation(
                        out=pt,
                        in_=ps,
                        func=AF.Exp,
                        bias=rcol_sb[:, mc : mc + 1],
                        scale=1.0,
                    )
                    pt_tiles.append(pt)
                    if mc == 1:
                        nc.vector.tensor_add(
                            out=acc,
                            in0=pt_tiles[0].bitcast(F32),
                            in1=pt_tiles[1].bitcast(F32),
                        )
                    elif mc > 1:
                        nc.vector.tensor_add(
                            out=acc,
                            in0=acc.bitcast(F32),
                            in1=pt.bitcast(F32),
                        )

                # denominator: single M=1 ones-matmul over the DVE partial sum
                ps_d = ps_den.tile([1, 512], F32, tag="den")
                nc.tensor.matmul(
                    ps_d, ones[:, :], acc[:, :], start=True, stop=True
                )
                den_sb = small.tile([1, 512], F32, tag="densb")
                nc.vector.tensor_copy(out=den_sb, in_=ps_d)
                nc.sync.dma_start(out=recip_dram[nb], in_=den_sb)
                recip_bc = small.tile([P, 512], F32, tag="recip_bc")
                rd = recip_dram[nb]
                nc.sync.dma_start(
                    out=recip_bc,
                    in_=bass.AP(
                        tensor=rd.tensor, offset=rd.offset,
                        ap=[[0, P], rd.ap[-1]],
                    ),
                )
                nc.vector.reciprocal(out=recip_bc, in_=recip_bc)

                # Y^T = VP^T . Pt, then normalize + bias + store
                for oc in range(DCH):
                    ps = ps_y.tile([P, 512], F32, tag="y")
                    for mc in range(NCH):
                        nc.tensor.matmul(
                            ps,
                            vp_sb[:, mc, ds(oc * P, P)],
                            pt_tiles[mc][:, :],
                            start=(mc == 0),
                            stop=(mc == NCH - 1),
                        )
                    t = outp.tile([P, 512], F32, tag="out")
                    nc.vector.tensor_tensor(
                        out=t, in0=ps, in1=recip_bc, op=mybir.AluOpType.mult
                    )
                    nc.vector.tensor_scalar_add(
                        out=t, in0=t, scalar1=bp_sb[:, oc : oc + 1]
                    )
                    nc.sync.dma_start(out=y[ds(oc * P, P), nsl], in_=t)

    split_sync_waits(nc)
    return nc


import numpy as np
from concourse.bass_utils import run_bass_kernel_spmd

B = 8

_NC_CACHE = None


def _get_nc():
    global _NC_CACHE
    if _NC_CACHE is None:
        _NC_CACHE = build_nc()
    return _NC_CACHE


def _make_in_maps(inputs):
    x = np.asarray(inputs["x"], np.float32)
    W_qkv = np.asarray(inputs["W_qkv"], np.float64)
    b_qkv = np.asarray(inputs["b_qkv"], np.float64)
    W_proj = np.asarray(inputs["W_proj"], np.float64)
    b_proj = np.asarray(inputs["b_proj"], np.float64)

    s = 1.0 / np.sqrt(np.float64(D))
    wq_s = W_qkv[:, :D] * s
    bq_s = b_qkv[:D] * s
    wk = W_qkv[:, D : 2 * D]
    wv = W_qkv[:, 2 * D :]
    bv = b_qkv[2 * D :]

    shared = {
        "wb": np.ascontiguousarray(
            (wq_s @ wk.T).astype(np.float32)
            .reshape(4, 128, 4, 128).transpose(2, 1, 0, 3)
        ),
        "wvp": np.ascontiguousarray((wv @ W_proj).astype(np.float32)),
        "u": np.ascontiguousarray((wk @ bq_s).astype(np.float32)),
        "bvp": np.ascontiguousarray((bv @ W_proj).astype(np.float32)),
        "bp": np.ascontiguousarray(b_proj.astype(np.float32)),
        "ones": np.ones((P, 1), np.float32),
    }
    return [{"x": np.ascontiguousarray(x[b]), **shared} for b in range(B)]


def kernel(**inputs):
    nc = _get_nc()
    in_maps = _make_in_maps(inputs)
    res = run_bass_kernel_spmd(nc, in_maps, core_ids=list(range(B)))
    return np.stack([res.results[b]["y"] for b in range(B)]).astype(np.float32)



# revision 7
# speedup vs baseline: 1.3471x; 1.3471x over previous
"""CrossVariableAttention Bass/Tile kernel for TRN2 — fp8 DoubleRow version.

Per-core program (data parallel over batch, one batch element per core).
Host-side algebraic fusions (see baseline): with B := 64*(Wq'.Wk^T),
u := 1024*(Wk.bq'), Wvp := Wv.Wp, bvp := bv.Wp:

  C' = B^T.X            (fp32r matmul, psum fp32 -> cast fp8 e4m3)  [= 64*C]
  r' = u8^T.x8          (fp8 DoubleRow, M=1)                        [= 1024*r]
  VP = X^T.Wvp + bvp    (fp32r matmul + DVE bias add -> fp8 e4m3)
  S' = x8^T.c8          (fp8 DoubleRow; 2 insts of K=256 per m-chunk)
  Pt = exp(S'/64 + r)   (ACT, scale=2^-6 folds the B-scale, bias=r per part.)
  den = ones8^T.Pt8     (fp8 DoubleRow, M=1, accumulated over 8 pairs)
  Y' = VP8^T.Pt8        (fp8 DoubleRow)
  y  = Y'*(1/den) + bp  (DVE; reciprocal_approx_fast; bf16 output)

fp8 e4m3 everywhere needs power-of-2 pre-scales chosen so tensors sit in
the normal range (WB*2^6 -> C' ~ N(0,0.94); u*2^10 -> r' ~ N(0,0.67));
the scales fold exactly into the ACT exp scale and the r' copy-back.
Measured numpy emulation of this exact pipeline: rel_err 1.5e-2 (gate 2e-2).
"""

from contextlib import ExitStack

import concourse.bass as bass
import concourse.mybir as mybir
import concourse.tile as tile
from concourse.bass import ds
from concourse.vector_clock import ScopedClock

F32 = mybir.dt.float32
F32R = mybir.dt.float32r
F8 = mybir.dt.float8e4
BF16 = mybir.dt.bfloat16
AF = mybir.ActivationFunctionType
DR = mybir.MatmulPerfMode.DoubleRow

P = 128
D = 512
N = 2048
DCH = D // P         # 4 d chunks
NCH = N // P         # 16 token chunks (m)
NSLAB = N // 512     # 4 query slabs

SC_B = 64.0          # 2^6 host scale on WB
SC_U = 1024.0        # 2^10 host scale on u


# ---------------------------------------------------------------------------
# The walrus build in this env accepts at most ONE sync wait per instruction
# (setupSyncWait: "Too many sync wait commands").  Tile attaches several.
# Fix: split excess waits onto engine-local NOPs placed just before the
# instruction (same engine => same stream order => identical semantics).
MAX_WAITS_PER_INST = 1


class SplitDrainTileContext(tile.TileContext):
    def _drain_and_barrier(self, tick_clock, wait_clock):
        nc = self.nc
        probe = nc.sync.nop(nofuse=True, hint="split_drain_waits")
        wait_clock.add_sem_waits(
            probe.ins, ScopedClock({None: tick_clock.global_clock})
        )
        waits = list(probe.ins.sync_info.on_wait)
        probe.ins.sync_info.on_wait = waits[:MAX_WAITS_PER_INST]
        for i in range(MAX_WAITS_PER_INST, len(waits), MAX_WAITS_PER_INST):
            extra = nc.sync.nop(nofuse=True, hint="split_drain_waits")
            extra.ins.sync_info = mybir.SyncInfo(
                on_wait=waits[i : i + MAX_WAITS_PER_INST], on_update=[]
            )
        nc.sync.drain()
        nc.all_engine_barrier()
        assert self.sems is not None
        popped = nc._tile_sem_poison_stack.pop()
        assert popped is self._sem_poison
        nc.clear_and_free_semaphores(list(self.sems.allocated().values()))
        nc.all_engine_barrier()


def split_sync_waits(nc, max_waits=MAX_WAITS_PER_INST):
    n_split = 0
    for fn in nc.m.functions:
        for bb in fn.blocks:
            insts = list(bb.instructions)
            out = []
            changed = False
            for inst in insts:
                si = getattr(inst, "sync_info", None)
                if si is not None:
                    waits = list(si.on_wait or [])
                    if len(waits) > max_waits:
                        changed = True
                        for j, w in enumerate(waits[: len(waits) - max_waits]):
                            out.append(
                                mybir.InstNoOp(
                                    name=f"{inst.name}-sw{j}",
                                    engine=inst.engine,
                                    bass_nofuse=True,
                                    sync_info=mybir.SyncInfo(
                                        on_wait=[w], on_update=[]
                                    ),
                                )
                            )
                            n_split += 1
                        si.on_wait = waits[len(waits) - max_waits :]
                out.append(inst)
            if changed:
                bb.instructions = out
    return n_split


def build_nc():
    nc = bass.Bass()

    x32 = nc.declare_dram_parameter("x32", [D, N], F32R, isOutput=False)
    x8 = nc.declare_dram_parameter("x8", [P, DCH, N], F8, isOutput=False)
    wb = nc.declare_dram_parameter("wb", [DCH, P, DCH, P], F32R, isOutput=False)
    wvp = nc.declare_dram_parameter("wvp", [D, D], F32R, isOutput=False)
    # padded to 16 cols: DoubleRow ldweights needs pair-dim step % 16 == 0
    u8_in = nc.declare_dram_parameter("u8", [P, DCH, 16], F8, isOutput=False)
    bvp = nc.declare_dram_parameter("bvp", [D], F32, isOutput=False)
    bp = nc.declare_dram_parameter("bp", [D], F32, isOutput=False)
    y = nc.declare_dram_parameter("y", [D, N], BF16, isOutput=True)
    r_dram = nc.dram_tensor("r_scratch", [N], F32)
    recip_dram = nc.dram_tensor("recip_scratch", [NSLAB, 512], F32)

    with SplitDrainTileContext(nc) as tc, ExitStack() as ctx:
        consts = ctx.enter_context(tc.tile_pool(name="consts", bufs=1))
        big = ctx.enter_context(tc.tile_pool(name="big", bufs=1))
        small = ctx.enter_context(tc.tile_pool(name="small", bufs=3))
        pt_pool = ctx.enter_context(tc.tile_pool(name="pt", bufs=2))
        rc_pool = ctx.enter_context(tc.tile_pool(name="rc", bufs=2))
        outp = ctx.enter_context(tc.tile_pool(name="outp", bufs=4))
        ps_big = ctx.enter_context(tc.tile_pool(name="ps_big", bufs=2, space="PSUM"))
        ps_s = ctx.enter_context(tc.tile_pool(name="ps_s", bufs=3, space="PSUM"))
        ps_small = ctx.enter_context(
            tc.tile_pool(name="ps_small", bufs=1, space="PSUM")
        )

        bp_sb = consts.tile([P, DCH], F32, tag="bp")
        u8_sb = consts.tile([P, DCH, 16], F8, tag="u8")
        ones8 = consts.tile([P, 2, 16], F8, tag="ones8")
        bvp_bc = consts.tile([P, D], F32, tag="bvp")
        wvp_sb = consts.tile([P, DCH, D], F32R, tag="wvp")
        rcol_sb = consts.tile([P, NCH], F32, tag="rcol")

        # --- persistent big tensors --------------------------------------
        c8_sb = big.tile([P, DCH, N], F8, tag="c8")
        x8_sb = big.tile([P, DCH, N], F8, tag="x8")
        vp8_sb = big.tile([P, NCH, D], F8, tag="vp8")
        x_tiles = []
        for nb in range(NSLAB):
            xt_nb = big.tile([P, DCH, 512], F32R, tag=f"x{nb}")
            x_tiles.append(xt_nb)

        # --- input DMAs ---------------------------------------------------
        wb_tiles = []
        for oc in range(DCH):
            wbt = consts.tile([P, DCH, P], F32R, tag=f"wb{oc}")
            wb_tiles.append(wbt)
        nc.sync.dma_start(out=wb_tiles[0], in_=wb[0])
        x_re = x32.rearrange("(c p) n -> p c n", p=P)
        nc.sync.dma_start(out=x_tiles[0], in_=x_re[:, :, ds(0, 512)])
        for oc in range(1, DCH):
            nc.sync.dma_start(out=wb_tiles[oc], in_=wb[oc])
        nc.sync.dma_start(out=x8_sb, in_=x8[:, :, :])
        nc.sync.dma_start(out=u8_sb, in_=u8_in[:, :, :])
        for nb in range(1, NSLAB):
            nc.sync.dma_start(out=x_tiles[nb], in_=x_re[:, :, ds(nb * 512, 512)])
        nc.sync.dma_start(out=wvp_sb, in_=wvp.rearrange("(c p) o -> p c o", p=P))
        nc.sync.dma_start(out=bp_sb, in_=bp.rearrange("(c p) -> p c", p=P))
        nc.gpsimd.memset(ones8, 1.0)
        bvp_ap = bvp[:]
        nc.sync.dma_start(
            out=bvp_bc,
            in_=bass.AP(
                tensor=bvp_ap.tensor, offset=bvp_ap.offset,
                ap=[[0, P], bvp_ap.ap[0]],
            ),
        )

        # --- phase A: C' (+fp8 cast) and r' per slab ----------------------
        for nb in range(NSLAB):
            nsl = ds(nb * 512, 512)
            for oc in range(DCH):
                ps = ps_big.tile([P, 512], F32, tag="psbig")
                for ic in range(DCH):
                    nc.tensor.matmul(
                        ps,
                        wb_tiles[oc][:, ic, :],
                        x_tiles[nb][:, ic, :],
                        start=(ic == 0),
                        stop=(ic == DCH - 1),
                    )
                nc.scalar.copy(out=c8_sb[:, oc, nsl], in_=ps)
            # r' = u8^T.x8 (DoubleRow, M=1), back to fp32 with 2^-10
            psr = ps_small.tile([1, 512], F32, tag="psr")
            for c2 in range(2):
                nc.tensor.matmul(
                    psr,
                    u8_sb[:, ds(2 * c2, 2), ds(0, 1)],
                    x8_sb[:, ds(2 * c2, 2), nsl],
                    start=(c2 == 0),
                    stop=(c2 == 1),
                    perf_mode=DR,
                )
            r_sb = small.tile([1, 512], F32, tag="rsb")
            nc.scalar.activation(
                out=r_sb, in_=psr, func=AF.Copy, scale=1.0 / SC_U
            )
            nc.sync.dma_start(out=r_dram[ds(nb * 512, 512)], in_=r_sb)

        # r in column layout [128, 16]: rcol[p, mc] = r[mc*128 + p]
        nc.sync.dma_start(
            out=rcol_sb, in_=r_dram.rearrange("(c p) -> p c", p=P)
        )

        def emit_s_exp(nb, pt8):
            """S'^T = x8^T.c8 for slab nb (DoubleRow) + exp -> pt8."""
            nsl = ds(nb * 512, 512)
            for mc in range(NCH):
                ps = ps_s.tile([P, 512], F32, tag="st")
                for c2 in range(2):
                    nc.tensor.matmul(
                        ps,
                        x8_sb[:, ds(2 * c2, 2), ds(mc * P, P)],
                        c8_sb[:, ds(2 * c2, 2), nsl],
                        start=(c2 == 0),
                        stop=(c2 == 1),
                        perf_mode=DR,
                    )
                nc.scalar.activation(
                    out=pt8[:, mc, :],
                    in_=ps,
                    func=AF.Exp,
                    bias=rcol_sb[:, mc : mc + 1],
                    scale=1.0 / SC_B,
                )

        # --- S'(0), then VP (VP's tensor time hides exp(0) latency) -------
        pt_tiles = []
        for _i in range(2):
            pt8_t = pt_pool.tile([P, NCH, 512], F8, tag="pt")
            pt_tiles.append(pt8_t)
        emit_s_exp(0, pt_tiles[0])

        for mc in range(NCH):
            ps = ps_big.tile([P, 512], F32, tag="psbig")
            for ic in range(DCH):
                nc.tensor.matmul(
                    ps,
                    x_tiles[mc // 4][:, ic, ds((mc % 4) * P, P)],
                    wvp_sb[:, ic, :],
                    start=(ic == 0),
                    stop=(ic == DCH - 1),
                )
            nc.vector.tensor_add(out=vp8_sb[:, mc, :], in0=ps, in1=bvp_bc)

        # --- phase B: attention per slab of 512 queries -------------------
        for nb in range(NSLAB):
            nsl = ds(nb * 512, 512)
            pt8 = pt_tiles[nb % 2]

            # den = ones8^T.Pt8 (DoubleRow, M=1, accumulate 8 pairs)
            ps_d = ps_small.tile([1, 512], F32, tag="den")
            for cm in range(NCH // 2):
                nc.tensor.matmul(
                    ps_d,
                    ones8[:, :, ds(0, 1)],
                    pt8[:, ds(2 * cm, 2), :],
                    start=(cm == 0),
                    stop=(cm == NCH // 2 - 1),
                    perf_mode=DR,
                )
            den_sb = small.tile([1, 512], F32, tag="densb")
            nc.vector.tensor_copy(out=den_sb, in_=ps_d)
            nc.sync.dma_start(out=recip_dram[nb], in_=den_sb)
            recip_bc = rc_pool.tile([P, 512], F32, tag="recip_bc")
            rd = recip_dram[nb]
            nc.sync.dma_start(
                out=recip_bc,
                in_=bass.AP(
                    tensor=rd.tensor, offset=rd.offset,
                    ap=[[0, P], rd.ap[-1]],
                ),
            )
            nc.vector.reciprocal(out=recip_bc, in_=recip_bc)

            # next slab's S' + exp overlap den/recip/Y' of this slab
            if nb + 1 < NSLAB:
                emit_s_exp(nb + 1, pt_tiles[(nb + 1) % 2])

            # Y'^T = VP8^T.Pt8 (DoubleRow), then normalize + bias + store
            for oc in range(DCH):
                ps = ps_big.tile([P, 512], F32, tag="psbig")
                for cm in range(NCH // 2):
                    nc.tensor.matmul(
                        ps,
                        vp8_sb[:, ds(2 * cm, 2), ds(oc * P, P)],
                        pt8[:, ds(2 * cm, 2), :],
                        start=(cm == 0),
                        stop=(cm == NCH // 2 - 1),
                        perf_mode=DR,
                    )
                t = outp.tile([P, 512], BF16, tag="out")
                nc.vector.tensor_tensor(
                    out=t, in0=ps, in1=recip_bc, op=mybir.AluOpType.mult
                )
                nc.vector.tensor_scalar_add(
                    out=t, in0=t, scalar1=bp_sb[:, oc : oc + 1]
                )
                nc.sync.dma_start(out=y[ds(oc * P, P), nsl], in_=t)

    split_sync_waits(nc)
    return nc


import ml_dtypes
import numpy as np
from concourse.bass_utils import run_bass_kernel_spmd

B = 8

_NC_CACHE = None


def _get_nc():
    global _NC_CACHE
    if _NC_CACHE is None:
        _NC_CACHE = build_nc()
    return _NC_CACHE


def _q8(a):
    return np.clip(a, -240, 240).astype(ml_dtypes.float8_e4m3fn)


def _make_in_maps(inputs):
    x = np.asarray(inputs["x"], np.float32)
    W_qkv = np.asarray(inputs["W_qkv"], np.float64)
    b_qkv = np.asarray(inputs["b_qkv"], np.float64)
    W_proj = np.asarray(inputs["W_proj"], np.float64)
    b_proj = np.asarray(inputs["b_proj"], np.float64)

    s = 1.0 / np.sqrt(np.float64(D))
    wq_s = W_qkv[:, :D] * s
    bq_s = b_qkv[:D] * s
    wk = W_qkv[:, D : 2 * D]
    wv = W_qkv[:, 2 * D :]
    bv = b_qkv[2 * D :]

    shared = {
        "wb": np.ascontiguousarray(
            (wq_s @ wk.T * SC_B).astype(np.float32)
            .reshape(4, 128, 4, 128).transpose(2, 1, 0, 3)
        ),
        "wvp": np.ascontiguousarray((wv @ W_proj).astype(np.float32)),
        "u8": np.ascontiguousarray(
            np.broadcast_to(
                _q8((wk @ bq_s * SC_U).astype(np.float32))
                .reshape(4, 128).T.reshape(128, 4, 1),
                (128, 4, 16),
            )
        ),
        "bvp": np.ascontiguousarray((bv @ W_proj).astype(np.float32)),
        "bp": np.ascontiguousarray(b_proj.astype(np.float32)),
    }
    return [
        {
            "x32": np.ascontiguousarray(x[b]),
            "x8": np.ascontiguousarray(
                _q8(x[b]).reshape(4, 128, N).transpose(1, 0, 2)
            ),
            **shared,
        }
        for b in range(B)
    ]


def kernel(**inputs):
    nc = _get_nc()
    in_maps = _make_in_maps(inputs)
    res = run_bass_kernel_spmd(nc, in_maps, core_ids=list(range(B)))
    return np.stack(
        [res.results[b]["y"].astype(np.float32) for b in range(B)]
    )


# revision 9
# speedup vs baseline: 1.5025x; 1.1154x over previous
"""CrossVariableAttention Bass/Tile kernel for TRN2 — fp8 DoubleRow version.

Per-core program (data parallel over batch, one batch element per core).
Host-side algebraic fusions (see baseline): with B := 64*(Wq'.Wk^T),
u := 1024*(Wk.bq'), Wvp := Wv.Wp, bvp := bv.Wp:

  C' = B^T.X            (fp32r matmul, psum fp32 -> cast fp8 e4m3)  [= 64*C]
  r' = u8^T.x8          (fp8 DoubleRow, M=1)                        [= 1024*r]
  VP = X^T.Wvp + bvp    (fp32r matmul + DVE bias add -> fp8 e4m3)
  S' = x8^T.c8          (fp8 DoubleRow; 2 insts of K=256 per m-chunk)
  Pt = exp(S'/64 + r)   (ACT, scale=2^-6 folds the B-scale, bias=r per part.)
  den = ones8^T.Pt8     (fp8 DoubleRow, M=1, accumulated over 8 pairs)
  Y' = VP8^T.Pt8        (fp8 DoubleRow)
  y  = Y'*(1/den) + bp  (DVE; reciprocal_approx_fast; bf16 output)

fp8 e4m3 everywhere needs power-of-2 pre-scales chosen so tensors sit in
the normal range (WB*2^6 -> C' ~ N(0,0.94); u*2^10 -> r' ~ N(0,0.67));
the scales fold exactly into the ACT exp scale and the r' copy-back.
Measured numpy emulation of this exact pipeline: rel_err 1.5e-2 (gate 2e-2).
"""

from contextlib import ExitStack

import concourse.bass as bass
import concourse.mybir as mybir
import concourse.tile as tile
from concourse.bass import ds
from concourse.vector_clock import ScopedClock

F32 = mybir.dt.float32
F32R = mybir.dt.float32r
F8 = mybir.dt.float8e4
BF16 = mybir.dt.bfloat16
AF = mybir.ActivationFunctionType
DR = mybir.MatmulPerfMode.DoubleRow

P = 128
D = 512
N = 2048
DCH = D // P         # 4 d chunks
NCH = N // P         # 16 token chunks (m)
NSLAB = N // 512     # 4 query slabs

SC_B = 64.0          # 2^6 host scale on WB
SC_U = 1024.0        # 2^10 host scale on u


# ---------------------------------------------------------------------------
# The walrus build in this env accepts at most ONE sync wait per instruction
# (setupSyncWait: "Too many sync wait commands").  Tile attaches several.
# Fix: split excess waits onto engine-local NOPs placed just before the
# instruction (same engine => same stream order => identical semantics).
MAX_WAITS_PER_INST = 1


class SplitDrainTileContext(tile.TileContext):
    def _drain_and_barrier(self, tick_clock, wait_clock):
        nc = self.nc
        probe = nc.sync.nop(nofuse=True, hint="split_drain_waits")
        wait_clock.add_sem_waits(
            probe.ins, ScopedClock({None: tick_clock.global_clock})
        )
        waits = list(probe.ins.sync_info.on_wait)
        probe.ins.sync_info.on_wait = waits[:MAX_WAITS_PER_INST]
        for i in range(MAX_WAITS_PER_INST, len(waits), MAX_WAITS_PER_INST):
            extra = nc.sync.nop(nofuse=True, hint="split_drain_waits")
            extra.ins.sync_info = mybir.SyncInfo(
                on_wait=waits[i : i + MAX_WAITS_PER_INST], on_update=[]
            )
        nc.sync.drain()
        nc.all_engine_barrier()
        assert self.sems is not None
        popped = nc._tile_sem_poison_stack.pop()
        assert popped is self._sem_poison
        nc.clear_and_free_semaphores(list(self.sems.allocated().values()))
        nc.all_engine_barrier()


def split_sync_waits(nc, max_waits=MAX_WAITS_PER_INST):
    n_split = 0
    for fn in nc.m.functions:
        for bb in fn.blocks:
            insts = list(bb.instructions)
            out = []
            changed = False
            for inst in insts:
                si = getattr(inst, "sync_info", None)
                if si is not None:
                    waits = list(si.on_wait or [])
                    if len(waits) > max_waits:
                        changed = True
                        for j, w in enumerate(waits[: len(waits) - max_waits]):
                            out.append(
                                mybir.InstNoOp(
                                    name=f"{inst.name}-sw{j}",
                                    engine=inst.engine,
                                    bass_nofuse=True,
                                    sync_info=mybir.SyncInfo(
                                        on_wait=[w], on_update=[]
                                    ),
                                )
                            )
                            n_split += 1
                        si.on_wait = waits[len(waits) - max_waits :]
                out.append(inst)
            if changed:
                bb.instructions = out
    return n_split


def build_nc():
    nc = bass.Bass()

    xb = nc.declare_dram_parameter("xb", [D, N], BF16, isOutput=False)
    x8 = nc.declare_dram_parameter("x8", [P, DCH, N], F8, isOutput=False)
    wb = nc.declare_dram_parameter("wb", [DCH, P, DCH, P], BF16, isOutput=False)
    wvp = nc.declare_dram_parameter("wvp", [D, D], BF16, isOutput=False)
    # padded to 16 cols: DoubleRow ldweights needs pair-dim step % 16 == 0
    u8_in = nc.declare_dram_parameter("u8", [P, DCH, 16], F8, isOutput=False)
    bvp = nc.declare_dram_parameter("bvp", [D], F32, isOutput=False)
    bp = nc.declare_dram_parameter("bp", [D], F32, isOutput=False)
    y = nc.declare_dram_parameter("y", [D, N], BF16, isOutput=True)
    r_dram = nc.dram_tensor("r_scratch", [N], F32)
    recip_dram = nc.dram_tensor("recip_scratch", [NSLAB, 512], F32)

    with SplitDrainTileContext(nc) as tc, ExitStack() as ctx:
        consts = ctx.enter_context(tc.tile_pool(name="consts", bufs=1))
        big = ctx.enter_context(tc.tile_pool(name="big", bufs=1))
        small = ctx.enter_context(tc.tile_pool(name="small", bufs=3))
        pt_pool = ctx.enter_context(tc.tile_pool(name="pt", bufs=2))
        rc_pool = ctx.enter_context(tc.tile_pool(name="rc", bufs=2))
        outp = ctx.enter_context(tc.tile_pool(name="outp", bufs=4))
        ps_big = ctx.enter_context(tc.tile_pool(name="ps_big", bufs=3, space="PSUM"))
        ps_s = ctx.enter_context(tc.tile_pool(name="ps_s", bufs=3, space="PSUM"))
        ps_small = ctx.enter_context(
            tc.tile_pool(name="ps_small", bufs=1, space="PSUM")
        )

        bp_sb = consts.tile([P, DCH], F32, tag="bp")
        u8_sb = consts.tile([P, DCH, 16], F8, tag="u8")
        ones8 = consts.tile([P, 2, 16], F8, tag="ones8")
        bvp_bc = consts.tile([P, D], F32, tag="bvp")
        wvp_sb = consts.tile([P, DCH, D], BF16, tag="wvp")
        rcol_sb = consts.tile([P, NCH], F32, tag="rcol")

        # --- persistent big tensors --------------------------------------
        c8_sb = big.tile([P, DCH, N], F8, tag="c8")
        x8_sb = big.tile([P, DCH, N], F8, tag="x8")
        vp8_sb = big.tile([P, NCH, D], F8, tag="vp8")
        x_tiles = []
        for nb in range(NSLAB):
            xt_nb = big.tile([P, DCH, 512], BF16, tag=f"x{nb}")
            x_tiles.append(xt_nb)

        # --- PE warmup: ~6us of dummy matmuls so the HAM clock-gate opens
        # (K=8/8 @ 2.4GHz) while the input DMAs are still in flight --------
        warm_w = consts.tile([P, P], BF16, tag="warmw")
        warm_x = consts.tile([P, 512], BF16, tag="warmx")
        nc.gpsimd.memset(warm_w, 0.0)
        nc.gpsimd.memset(warm_x, 0.0)
        ps_warm = ps_big.tile([P, 512], F32, tag="psbig")
        for _ in range(14):
            nc.tensor.matmul(
                ps_warm, warm_w, warm_x, start=True, stop=True,
                skip_group_check=True,
            )

        # --- input DMAs ---------------------------------------------------
        wb_tiles = []
        for oc in range(DCH):
            wbt = consts.tile([P, DCH, P], BF16, tag=f"wb{oc}")
            wb_tiles.append(wbt)
        nc.sync.dma_start(out=wb_tiles[0], in_=wb[0])
        x_re = xb.rearrange("(c p) n -> p c n", p=P)
        nc.sync.dma_start(out=x_tiles[0], in_=x_re[:, :, ds(0, 512)])
        for oc in range(1, DCH):
            nc.sync.dma_start(out=wb_tiles[oc], in_=wb[oc])
        nc.sync.dma_start(out=x8_sb, in_=x8[:, :, :])
        nc.sync.dma_start(out=u8_sb, in_=u8_in[:, :, :])
        for nb in range(1, NSLAB):
            nc.sync.dma_start(out=x_tiles[nb], in_=x_re[:, :, ds(nb * 512, 512)])
        nc.sync.dma_start(out=wvp_sb, in_=wvp.rearrange("(c p) o -> p c o", p=P))
        nc.sync.dma_start(out=bp_sb, in_=bp.rearrange("(c p) -> p c", p=P))
        nc.gpsimd.memset(ones8, 1.0)
        bvp_ap = bvp[:]
        nc.sync.dma_start(
            out=bvp_bc,
            in_=bass.AP(
                tensor=bvp_ap.tensor, offset=bvp_ap.offset,
                ap=[[0, P], bvp_ap.ap[0]],
            ),
        )

        # --- phase A: C' (+fp8 cast) and r' per slab ----------------------
        for nb in range(NSLAB):
            nsl = ds(nb * 512, 512)
            for oc in range(DCH):
                ps = ps_big.tile([P, 512], F32, tag="psbig")
                for ic in range(DCH):
                    nc.tensor.matmul(
                        ps,
                        wb_tiles[oc][:, ic, :],
                        x_tiles[nb][:, ic, :],
                        start=(ic == 0),
                        stop=(ic == DCH - 1),
                    )
                nc.scalar.copy(out=c8_sb[:, oc, nsl], in_=ps)
            # r' = u8^T.x8 (DoubleRow, M=1), back to fp32 with 2^-10
            psr = ps_small.tile([1, 512], F32, tag="small")
            for c2 in range(2):
                nc.tensor.matmul(
                    psr,
                    u8_sb[:, ds(2 * c2, 2), ds(0, 1)],
                    x8_sb[:, ds(2 * c2, 2), nsl],
                    start=(c2 == 0),
                    stop=(c2 == 1),
                    perf_mode=DR,
                )
            r_sb = small.tile([1, 512], F32, tag="rsb")
            nc.scalar.activation(
                out=r_sb, in_=psr, func=AF.Copy, scale=1.0 / SC_U
            )
            nc.sync.dma_start(out=r_dram[ds(nb * 512, 512)], in_=r_sb)

        # r in column layout [128, 16]: rcol[p, mc] = r[mc*128 + p]
        nc.sync.dma_start(
            out=rcol_sb, in_=r_dram.rearrange("(c p) -> p c", p=P)
        )

        def emit_s_mc(nb, mc, pt8):
            """One m-chunk of S'^T = x8^T.c8 (DoubleRow) + exp -> pt8."""
            nsl = ds(nb * 512, 512)
            ps = ps_s.tile([P, 512], F32, tag="st")
            for c2 in range(2):
                nc.tensor.matmul(
                    ps,
                    x8_sb[:, ds(2 * c2, 2), ds(mc * P, P)],
                    c8_sb[:, ds(2 * c2, 2), nsl],
                    start=(c2 == 0),
                    stop=(c2 == 1),
                    perf_mode=DR,
                )
            nc.scalar.activation(
                out=pt8[:, mc, :],
                in_=ps,
                func=AF.Exp,
                bias=rcol_sb[:, mc : mc + 1],
                scale=1.0 / SC_B,
            )

        def emit_s_exp(nb, pt8):
            for mc in range(NCH):
                emit_s_mc(nb, mc, pt8)

        # --- S'(0), then VP (VP's tensor time hides exp(0) latency) -------
        pt_tiles = []
        for _i in range(2):
            pt8_t = pt_pool.tile([P, NCH, 512], F8, tag="pt")
            pt_tiles.append(pt8_t)
        emit_s_exp(0, pt_tiles[0])

        for mc in range(NCH):
            ps = ps_big.tile([P, 512], F32, tag="psbig")
            for ic in range(DCH):
                nc.tensor.matmul(
                    ps,
                    x_tiles[mc // 4][:, ic, ds((mc % 4) * P, P)],
                    wvp_sb[:, ic, :],
                    start=(ic == 0),
                    stop=(ic == DCH - 1),
                )
            nc.vector.tensor_add(out=vp8_sb[:, mc, :], in0=ps, in1=bvp_bc)

        # --- phase B: attention per slab of 512 queries -------------------
        for nb in range(NSLAB):
            nsl = ds(nb * 512, 512)
            pt8 = pt_tiles[nb % 2]

            # den = ones8^T.Pt8 (DoubleRow, M=1, accumulate 8 pairs)
            ps_d = ps_small.tile([1, 512], F32, tag="small")
            for cm in range(NCH // 2):
                nc.tensor.matmul(
                    ps_d,
                    ones8[:, :, ds(0, 1)],
                    pt8[:, ds(2 * cm, 2), :],
                    start=(cm == 0),
                    stop=(cm == NCH // 2 - 1),
                    perf_mode=DR,
                )
            lg_sb = small.tile([1, 512], F32, tag="lg")
            nc.scalar.activation(out=lg_sb, in_=ps_d, func=AF.Ln)
            rc1_sb = small.tile([1, 512], F32, tag="rc1")
            nc.scalar.activation(out=rc1_sb, in_=lg_sb, func=AF.Exp, scale=-1.0)
            nc.sync.dma_start(out=recip_dram[nb], in_=rc1_sb)
            recip_bc = rc_pool.tile([P, 512], F32, tag="recip_bc")
            rd = recip_dram[nb]
            nc.sync.dma_start(
                out=recip_bc,
                in_=bass.AP(
                    tensor=rd.tensor, offset=rd.offset,
                    ap=[[0, P], rd.ap[-1]],
                ),
            )
            # Y'^T = VP8^T.Pt8 (DoubleRow), then normalize + bias + store.
            # Next slab's S'+exp is interleaved between this slab's Y' chunks
            # so exp (688ns/tile on ACT) paces under ~13.6us of tensor work
            # and the recip chain hides under the first S' chunk.
            for oc in range(DCH):
                if nb + 1 < NSLAB:
                    for mc in range(4 * oc, 4 * oc + 4):
                        emit_s_mc(nb + 1, mc, pt_tiles[(nb + 1) % 2])
                ps = ps_big.tile([P, 512], F32, tag="psbig")
                for cm in range(NCH // 2):
                    nc.tensor.matmul(
                        ps,
                        vp8_sb[:, ds(2 * cm, 2), ds(oc * P, P)],
                        pt8[:, ds(2 * cm, 2), :],
                        start=(cm == 0),
                        stop=(cm == NCH // 2 - 1),
                        perf_mode=DR,
                    )
                t = outp.tile([P, 512], BF16, tag="out")
                nc.vector.tensor_tensor(
                    out=t, in0=ps, in1=recip_bc, op=mybir.AluOpType.mult
                )
                nc.vector.tensor_scalar_add(
                    out=t, in0=t, scalar1=bp_sb[:, oc : oc + 1]
                )
                nc.sync.dma_start(out=y[ds(oc * P, P), nsl], in_=t)

    split_sync_waits(nc)
    return nc


import ml_dtypes
import numpy as np
from concourse.bass_utils import run_bass_kernel_spmd

B = 8

_NC_CACHE = None


def _get_nc():
    global _NC_CACHE
    if _NC_CACHE is None:
        _NC_CACHE = build_nc()
    return _NC_CACHE


def _q8(a):
    return np.clip(a, -240, 240).astype(ml_dtypes.float8_e4m3fn)


def _make_in_maps(inputs):
    x = np.asarray(inputs["x"], np.float32)
    W_qkv = np.asarray(inputs["W_qkv"], np.float64)
    b_qkv = np.asarray(inputs["b_qkv"], np.float64)
    W_proj = np.asarray(inputs["W_proj"], np.float64)
    b_proj = np.asarray(inputs["b_proj"], np.float64)

    s = 1.0 / np.sqrt(np.float64(D))
    wq_s = W_qkv[:, :D] * s
    bq_s = b_qkv[:D] * s
    wk = W_qkv[:, D : 2 * D]
    wv = W_qkv[:, 2 * D :]
    bv = b_qkv[2 * D :]

    shared = {
        "wb": np.ascontiguousarray(
            (wq_s @ wk.T * SC_B).astype(ml_dtypes.bfloat16)
            .reshape(4, 128, 4, 128).transpose(2, 1, 0, 3)
        ),
        "wvp": np.ascontiguousarray((wv @ W_proj).astype(ml_dtypes.bfloat16)),
        "u8": np.ascontiguousarray(
            np.broadcast_to(
                _q8((wk @ bq_s * SC_U).astype(np.float32))
                .reshape(4, 128).T.reshape(128, 4, 1),
                (128, 4, 16),
            )
        ),
        "bvp": np.ascontiguousarray((bv @ W_proj).astype(np.float32)),
        "bp": np.ascontiguousarray(b_proj.astype(np.float32)),
    }
    return [
        {
            "xb": np.ascontiguousarray(x[b].astype(ml_dtypes.bfloat16)),
            "x8": np.ascontiguousarray(
                _q8(x[b]).reshape(4, 128, N).transpose(1, 0, 2)
            ),
            **shared,
        }
        for b in range(B)
    ]


def kernel(**inputs):
    nc = _get_nc()
    in_maps = _make_in_maps(inputs)
    res = run_bass_kernel_spmd(nc, in_maps, core_ids=list(range(B)))
    return np.stack(
        [res.results[b]["y"].astype(np.float32) for b in range(B)]
    )


# revision 11
# speedup vs baseline: 1.5388x; 1.0242x over previous
"""CrossVariableAttention Bass/Tile kernel for TRN2 — fp8 DoubleRow version.

Per-core program (data parallel over batch, one batch element per core).
Host-side algebraic fusions (see baseline): with B := 64*(Wq'.Wk^T),
u := 1024*(Wk.bq'), Wvp := Wv.Wp, bvp := bv.Wp:

  C' = B^T.X            (fp32r matmul, psum fp32 -> cast fp8 e4m3)  [= 64*C]
  r' = u8^T.x8          (fp8 DoubleRow, M=1)                        [= 1024*r]
  VP = X^T.Wvp + bvp    (fp32r matmul + DVE bias add -> fp8 e4m3)
  S' = x8^T.c8          (fp8 DoubleRow; 2 insts of K=256 per m-chunk)
  Pt = exp(S'/64 + r)   (ACT, scale=2^-6 folds the B-scale, bias=r per part.)
  den = ones8^T.Pt8     (fp8 DoubleRow, M=1, accumulated over 8 pairs)
  Y' = VP8^T.Pt8        (fp8 DoubleRow)
  y  = Y'*(1/den) + bp  (DVE; reciprocal_approx_fast; bf16 output)

fp8 e4m3 everywhere needs power-of-2 pre-scales chosen so tensors sit in
the normal range (WB*2^6 -> C' ~ N(0,0.94); u*2^10 -> r' ~ N(0,0.67));
the scales fold exactly into the ACT exp scale and the r' copy-back.
Measured numpy emulation of this exact pipeline: rel_err 1.5e-2 (gate 2e-2).
"""

from contextlib import ExitStack

import concourse.bass as bass
import concourse.mybir as mybir
import concourse.tile as tile
from concourse.bass import ds
from concourse.vector_clock import ScopedClock

F32 = mybir.dt.float32
F32R = mybir.dt.float32r
F8 = mybir.dt.float8e4
BF16 = mybir.dt.bfloat16
AF = mybir.ActivationFunctionType
DR = mybir.MatmulPerfMode.DoubleRow

P = 128
D = 512
N = 2048
DCH = D // P         # 4 d chunks
NCH = N // P         # 16 token chunks (m)
NSLAB = N // 512     # 4 query slabs

SC_B = 64.0          # 2^6 host scale on WB
SC_U = 1024.0        # 2^10 host scale on u


# ---------------------------------------------------------------------------
# The walrus build in this env accepts at most ONE sync wait per instruction
# (setupSyncWait: "Too many sync wait commands").  Tile attaches several.
# Fix: split excess waits onto engine-local NOPs placed just before the
# instruction (same engine => same stream order => identical semantics).
MAX_WAITS_PER_INST = 1


class SplitDrainTileContext(tile.TileContext):
    def _drain_and_barrier(self, tick_clock, wait_clock):
        nc = self.nc
        probe = nc.sync.nop(nofuse=True, hint="split_drain_waits")
        wait_clock.add_sem_waits(
            probe.ins, ScopedClock({None: tick_clock.global_clock})
        )
        waits = list(probe.ins.sync_info.on_wait)
        probe.ins.sync_info.on_wait = waits[:MAX_WAITS_PER_INST]
        for i in range(MAX_WAITS_PER_INST, len(waits), MAX_WAITS_PER_INST):
            extra = nc.sync.nop(nofuse=True, hint="split_drain_waits")
            extra.ins.sync_info = mybir.SyncInfo(
                on_wait=waits[i : i + MAX_WAITS_PER_INST], on_update=[]
            )
        nc.sync.drain()
        nc.all_engine_barrier()
        assert self.sems is not None
        popped = nc._tile_sem_poison_stack.pop()
        assert popped is self._sem_poison
        nc.clear_and_free_semaphores(list(self.sems.allocated().values()))
        nc.all_engine_barrier()


def split_sync_waits(nc, max_waits=MAX_WAITS_PER_INST):
    n_split = 0
    for fn in nc.m.functions:
        for bb in fn.blocks:
            insts = list(bb.instructions)
            out = []
            changed = False
            for inst in insts:
                si = getattr(inst, "sync_info", None)
                if si is not None:
                    waits = list(si.on_wait or [])
                    if len(waits) > max_waits:
                        changed = True
                        for j, w in enumerate(waits[: len(waits) - max_waits]):
                            out.append(
                                mybir.InstNoOp(
                                    name=f"{inst.name}-sw{j}",
                                    engine=inst.engine,
                                    bass_nofuse=True,
                                    sync_info=mybir.SyncInfo(
                                        on_wait=[w], on_update=[]
                                    ),
                                )
                            )
                            n_split += 1
                        si.on_wait = waits[len(waits) - max_waits :]
                out.append(inst)
            if changed:
                bb.instructions = out
    return n_split


def build_nc():
    nc = bass.Bass()

    xb = nc.declare_dram_parameter("xb", [D, N], BF16, isOutput=False)
    x8 = nc.declare_dram_parameter("x8", [P, DCH, N], F8, isOutput=False)
    wb = nc.declare_dram_parameter("wb", [DCH, P, DCH, P], BF16, isOutput=False)
    wvp = nc.declare_dram_parameter("wvp", [D, D], BF16, isOutput=False)
    # padded to 16 cols: DoubleRow ldweights needs pair-dim step % 16 == 0
    u8_in = nc.declare_dram_parameter("u8", [P, DCH, 16], F8, isOutput=False)
    bvp = nc.declare_dram_parameter("bvp", [D], F32, isOutput=False)
    bp = nc.declare_dram_parameter("bp", [D], F32, isOutput=False)
    y = nc.declare_dram_parameter("y", [D, N], BF16, isOutput=True)
    r_dram = nc.dram_tensor("r_scratch", [N], F32)
    recip_dram = nc.dram_tensor("recip_scratch", [NSLAB, 512], F32)

    with SplitDrainTileContext(nc) as tc, ExitStack() as ctx:
        consts = ctx.enter_context(tc.tile_pool(name="consts", bufs=1))
        big = ctx.enter_context(tc.tile_pool(name="big", bufs=1))
        small = ctx.enter_context(tc.tile_pool(name="small", bufs=3))
        pt_pool = ctx.enter_context(tc.tile_pool(name="pt", bufs=2))
        rc_pool = ctx.enter_context(tc.tile_pool(name="rc", bufs=2))
        outp = ctx.enter_context(tc.tile_pool(name="outp", bufs=4))
        ps_big = ctx.enter_context(tc.tile_pool(name="ps_big", bufs=3, space="PSUM"))
        ps_s = ctx.enter_context(tc.tile_pool(name="ps_s", bufs=3, space="PSUM"))
        ps_small = ctx.enter_context(
            tc.tile_pool(name="ps_small", bufs=1, space="PSUM")
        )

        bp_sb = consts.tile([P, DCH], F32, tag="bp")
        u8_sb = consts.tile([P, DCH, 16], F8, tag="u8")
        ones8 = consts.tile([P, 2, 16], F8, tag="ones8")
        bvp_bc = consts.tile([P, D], F32, tag="bvp")
        wvp_sb = consts.tile([P, DCH, D], BF16, tag="wvp")
        rcol_sb = consts.tile([P, NCH], F32, tag="rcol")

        # --- persistent big tensors --------------------------------------
        c8_sb = big.tile([P, DCH, N], F8, tag="c8")
        x8_sb = big.tile([P, DCH, N], F8, tag="x8")
        vp8_sb = big.tile([P, NCH, D], F8, tag="vp8")
        x_tiles = []
        for nb in range(NSLAB):
            xt_nb = big.tile([P, DCH, 512], BF16, tag=f"x{nb}")
            x_tiles.append(xt_nb)

        # --- PE warmup: ~6us of dummy matmuls so the HAM clock-gate opens
        # (K=8/8 @ 2.4GHz) while the input DMAs are still in flight --------
        warm_w = consts.tile([P, P], BF16, tag="warmw")
        warm_x = consts.tile([P, 512], BF16, tag="warmx")
        nc.gpsimd.memset(warm_w, 0.0)
        nc.gpsimd.memset(warm_x, 0.0)
        ps_warm = ps_big.tile([P, 512], F32, tag="psbig")
        for _ in range(14):
            nc.tensor.matmul(
                ps_warm, warm_w, warm_x, start=True, stop=True,
                skip_group_check=True,
            )

        # --- input DMAs ---------------------------------------------------
        wb_tiles = []
        for oc in range(DCH):
            wbt = consts.tile([P, DCH, P], BF16, tag=f"wb{oc}")
            wb_tiles.append(wbt)
        nc.sync.dma_start(out=wb_tiles[0], in_=wb[0])
        x_re = xb.rearrange("(c p) n -> p c n", p=P)
        nc.sync.dma_start(out=x_tiles[0], in_=x_re[:, :, ds(0, 512)])
        for oc in range(1, DCH):
            nc.sync.dma_start(out=wb_tiles[oc], in_=wb[oc])
        for nb in range(1, NSLAB):
            nc.sync.dma_start(out=x_tiles[nb], in_=x_re[:, :, ds(nb * 512, 512)])
        nc.sync.dma_start(out=x8_sb, in_=x8[:, :, :])
        nc.sync.dma_start(out=u8_sb, in_=u8_in[:, :, :])
        nc.sync.dma_start(out=wvp_sb, in_=wvp.rearrange("(c p) o -> p c o", p=P))
        nc.sync.dma_start(out=bp_sb, in_=bp.rearrange("(c p) -> p c", p=P))
        nc.gpsimd.memset(ones8, 1.0)
        ones_col = consts.tile([1, P], F32, tag="ones_col")
        nc.gpsimd.memset(ones_col, 1.0)
        bvp_ap = bvp[:]
        nc.sync.dma_start(
            out=bvp_bc,
            in_=bass.AP(
                tensor=bvp_ap.tensor, offset=bvp_ap.offset,
                ap=[[0, P], bvp_ap.ap[0]],
            ),
        )

        # --- phase A: C' (+fp8 cast) per slab, then r' ---------------------
        for nb in range(NSLAB):
            nsl = ds(nb * 512, 512)
            for oc in range(DCH):
                ps = ps_big.tile([P, 512], F32, tag="psbig")
                for ic in range(DCH):
                    nc.tensor.matmul(
                        ps,
                        wb_tiles[oc][:, ic, :],
                        x_tiles[nb][:, ic, :],
                        start=(ic == 0),
                        stop=(ic == DCH - 1),
                    )
                nc.scalar.copy(out=c8_sb[:, oc, nsl], in_=ps)
        for nb in range(NSLAB):
            # r' = u8^T.x8 (DoubleRow, M=1), back to fp32 with 2^-10
            nsl = ds(nb * 512, 512)
            psr = ps_small.tile([1, 512], F32, tag="small")
            for c2 in range(2):
                nc.tensor.matmul(
                    psr,
                    u8_sb[:, ds(2 * c2, 2), ds(0, 1)],
                    x8_sb[:, ds(2 * c2, 2), nsl],
                    start=(c2 == 0),
                    stop=(c2 == 1),
                    perf_mode=DR,
                )
            r_sb = small.tile([1, 512], F32, tag="rsb")
            nc.scalar.activation(
                out=r_sb, in_=psr, func=AF.Copy, scale=1.0 / SC_U
            )
            nc.sync.dma_start(out=r_dram[ds(nb * 512, 512)], in_=r_sb)

        # r in column layout [128, 16]: rcol[p, mc] = r[mc*128 + p]
        nc.sync.dma_start(
            out=rcol_sb, in_=r_dram.rearrange("(c p) -> p c", p=P)
        )

        def emit_s_mc(nb, mc, pt8):
            """One m-chunk of S'^T = x8^T.c8 (DoubleRow) + exp -> pt8."""
            nsl = ds(nb * 512, 512)
            ps = ps_s.tile([P, 512], F32, tag="st")
            for c2 in range(2):
                nc.tensor.matmul(
                    ps,
                    x8_sb[:, ds(2 * c2, 2), ds(mc * P, P)],
                    c8_sb[:, ds(2 * c2, 2), nsl],
                    start=(c2 == 0),
                    stop=(c2 == 1),
                    perf_mode=DR,
                )
            nc.scalar.activation(
                out=pt8[:, mc, :],
                in_=ps,
                func=AF.Exp,
                bias=rcol_sb[:, mc : mc + 1],
                scale=1.0 / SC_B,
            )

        def emit_s_exp(nb, pt8):
            for mc in range(NCH):
                emit_s_mc(nb, mc, pt8)

        # --- S'(0), then VP (VP's tensor time hides exp(0) latency) -------
        pt_tiles = []
        for _i in range(2):
            pt8_t = pt_pool.tile([P, NCH, 512], F8, tag="pt")
            pt_tiles.append(pt8_t)
        emit_s_exp(0, pt_tiles[0])

        for mc in range(NCH):
            ps = ps_big.tile([P, 512], F32, tag="psbig")
            for ic in range(DCH):
                nc.tensor.matmul(
                    ps,
                    x_tiles[mc // 4][:, ic, ds((mc % 4) * P, P)],
                    wvp_sb[:, ic, :],
                    start=(ic == 0),
                    stop=(ic == DCH - 1),
                )
            nc.vector.tensor_add(out=vp8_sb[:, mc, :], in0=ps, in1=bvp_bc)

        # --- phase B: attention per slab of 512 queries -------------------
        for nb in range(NSLAB):
            nsl = ds(nb * 512, 512)
            pt8 = pt_tiles[nb % 2]

            # den = ones8^T.Pt8 (DoubleRow, M=1, accumulate 8 pairs)
            ps_d = ps_small.tile([1, 512], F32, tag="small")
            for cm in range(NCH // 2):
                nc.tensor.matmul(
                    ps_d,
                    ones8[:, :, ds(0, 1)],
                    pt8[:, ds(2 * cm, 2), :],
                    start=(cm == 0),
                    stop=(cm == NCH // 2 - 1),
                    perf_mode=DR,
                )
            lg_sb = small.tile([1, 512], F32, tag="lg")
            nc.scalar.activation(out=lg_sb, in_=ps_d, func=AF.Ln)
            rc1_sb = small.tile([1, 512], F32R, tag="rc1")
            nc.scalar.activation(out=rc1_sb, in_=lg_sb, func=AF.Exp, scale=-1.0)

            # next slab's first S' chunk covers the Ln/Exp latency
            if nb + 1 < NSLAB:
                for mc in range(4):
                    emit_s_mc(nb + 1, mc, pt_tiles[(nb + 1) % 2])

            # broadcast 1/den across partitions with a K=1 matmul
            ps_bc = ps_small.tile([P, 512], F32, tag="bc")
            nc.tensor.matmul(
                ps_bc, ones_col.bitcast(F32R), rc1_sb, start=True, stop=True
            )
            recip_bc = rc_pool.tile([P, 512], F32, tag="recip_bc")
            nc.scalar.copy(out=recip_bc, in_=ps_bc)

            # Y'^T = VP8^T.Pt8 (DoubleRow), then normalize + bias + store.
            # Next slab's S'+exp is interleaved between this slab's Y' chunks
            # so exp (688ns/tile on ACT) paces under ~13.6us of tensor work.
            for oc in range(DCH):
                if nb + 1 < NSLAB and oc > 0:
                    for mc in range(4 * oc, 4 * oc + 4):
                        emit_s_mc(nb + 1, mc, pt_tiles[(nb + 1) % 2])
                ps = ps_big.tile([P, 512], F32, tag="psbig")
                for cm in range(NCH // 2):
                    nc.tensor.matmul(
                        ps,
                        vp8_sb[:, ds(2 * cm, 2), ds(oc * P, P)],
                        pt8[:, ds(2 * cm, 2), :],
                        start=(cm == 0),
                        stop=(cm == NCH // 2 - 1),
                        perf_mode=DR,
                    )
                t = outp.tile([P, 512], BF16, tag="out")
                nc.vector.tensor_tensor(
                    out=t, in0=ps, in1=recip_bc, op=mybir.AluOpType.mult
                )
                nc.vector.tensor_scalar_add(
                    out=t, in0=t, scalar1=bp_sb[:, oc : oc + 1]
                )
                nc.sync.dma_start(out=y[ds(oc * P, P), nsl], in_=t)

    split_sync_waits(nc)
    return nc


import ml_dtypes
import numpy as np
from concourse.bass_utils import run_bass_kernel_spmd

B = 8

_NC_CACHE = None


def _get_nc():
    global _NC_CACHE
    if _NC_CACHE is None:
        _NC_CACHE = build_nc()
    return _NC_CACHE


def _q8(a):
    return np.clip(a, -240, 240).astype(ml_dtypes.float8_e4m3fn)


def _make_in_maps(inputs):
    x = np.asarray(inputs["x"], np.float32)
    W_qkv = np.asarray(inputs["W_qkv"], np.float64)
    b_qkv = np.asarray(inputs["b_qkv"], np.float64)
    W_proj = np.asarray(inputs["W_proj"], np.float64)
    b_proj = np.asarray(inputs["b_proj"], np.float64)

    s = 1.0 / np.sqrt(np.float64(D))
    wq_s = W_qkv[:, :D] * s
    bq_s = b_qkv[:D] * s
    wk = W_qkv[:, D : 2 * D]
    wv = W_qkv[:, 2 * D :]
    bv = b_qkv[2 * D :]

    shared = {
        "wb": np.ascontiguousarray(
            (wq_s @ wk.T * SC_B).astype(ml_dtypes.bfloat16)
            .reshape(4, 128, 4, 128).transpose(2, 1, 0, 3)
        ),
        "wvp": np.ascontiguousarray((wv @ W_proj).astype(ml_dtypes.bfloat16)),
        "u8": np.ascontiguousarray(
            np.broadcast_to(
                _q8((wk @ bq_s * SC_U).astype(np.float32))
                .reshape(4, 128).T.reshape(128, 4, 1),
                (128, 4, 16),
            )
        ),
        "bvp": np.ascontiguousarray((bv @ W_proj).astype(np.float32)),
        "bp": np.ascontiguousarray(b_proj.astype(np.float32)),
    }
    return [
        {
            "xb": np.ascontiguousarray(x[b].astype(ml_dtypes.bfloat16)),
            "x8": np.ascontiguousarray(
                _q8(x[b]).reshape(4, 128, N).transpose(1, 0, 2)
            ),
            **shared,
        }
        for b in range(B)
    ]


def kernel(**inputs):
    nc = _get_nc()
    in_maps = _make_in_maps(inputs)
    res = run_bass_kernel_spmd(nc, in_maps, core_ids=list(range(B)))
    return np.stack(
        [res.results[b]["y"].astype(np.float32) for b in range(B)]
    )
